# revision 1
# baseline (speedup 1.0000x reference)
"""GQA attention kernel for 8 Trainium2 NeuronCores.

Sharding: tensor-parallel over heads. Core i handles query heads (2i, 2i+1)
and KV head i//2. Out-proj is row-parallel: each core emits a partial
[S, DIM] output; the host sums the 8 partials and adds the output bias.

On-chip layouts keep head_dim (128) on partitions and sequence on the free
axis, so QK^T needs no transposes, softmax statistics are PE ones-matmuls,
and the attention weights feed the AV matmul directly from the exp output.
"""

import numpy as np

DIM = 2048
Q_HEADS = 16
KV_HEADS = 4
HEAD_DIM = 128
S = 2048
MAX_LEN = 2048
ROPE_THETA = 10000.0
ROPE_FACTOR = 8.0
N_CORES = 8
HEADS_PER_CORE = Q_HEADS // N_CORES  # 2
SCALE = 1.0 / np.sqrt(HEAD_DIM)
NEG = -1.0e30

_F32R_CACHE = {}


def _rope_cos_sin_T():
    d = HEAD_DIM
    seq_eff = max(S, MAX_LEN)
    base_adj = (ROPE_FACTOR * seq_eff / MAX_LEN - (ROPE_FACTOR - 1.0)) ** (d / (d - 2))
    adjusted_base = ROPE_THETA * base_adj
    inv_freq = 1.0 / adjusted_base ** (np.arange(0, d, 2, dtype=np.float32) / d)
    pos = np.arange(S, dtype=np.float32)
    freqs = pos[:, None] * inv_freq[None, :]
    emb = np.concatenate([freqs, freqs], axis=-1)  # [S, d]
    return (
        np.ascontiguousarray(np.cos(emb).T.astype(np.float32)),  # [d, S]
        np.ascontiguousarray(np.sin(emb).T.astype(np.float32)),
    )


def _masks():
    # additive masks for the 4 diagonal 128x512 blocks: block r covers keys
    # [128r, 128r+128) against queries [0, 512) within a 512-query chunk.
    k = np.arange(128)[:, None]
    q = np.arange(512)[None, :]
    m = np.zeros((128, 4, 512), np.float32)
    for r in range(4):
        m[:, r, :] = np.where(128 * r + k > q, NEG, 0.0).astype(np.float32)
    return np.ascontiguousarray(m.reshape(128, 4 * 512))


def _build_program():
    import concourse.bass as bass
    import concourse.tile as tile
    from concourse import mybir
    import bass_rust
    from concourse.vector_clock import ScopedClock
    from concourse.masks import make_identity

    # --- workaround: walrus CTRL instructions accept a single sync wait;
    # split the TileContext end-drain waits across one SP nop each.
    def _patched_drain_and_barrier(self, tick_clock, wait_clock):
        nop0 = self.nc.sync.nop(nofuse=True)
        wait_clock.add_sem_waits(nop0.ins, ScopedClock({None: tick_clock.global_clock}))
        si = nop0.ins.sync_info
        ws = list(si.on_wait) if si is not None else []
        if len(ws) > 1:
            nop0.ins.sync_info = bass_rust.SyncInfo(
                on_wait=ws[:1], on_update=list(si.on_update))
            for i in range(1, len(ws)):
                nop = self.nc.sync.nop(nofuse=True)
                nop.ins.sync_info = bass_rust.SyncInfo(on_wait=ws[i:i + 1], on_update=[])
        self.nc.sync.drain()
        self.nc.all_engine_barrier()
        popped = self.nc._tile_sem_poison_stack.pop()
        assert popped is self._sem_poison
        self.nc.clear_and_free_semaphores(list(self.sems.allocated().values()))
        self.nc.all_engine_barrier()

    tile.TileContext._drain_and_barrier = _patched_drain_and_barrier

    def _split_multi_waits(nc):
        # this walrus build accepts a single sync-wait slot on several
        # instruction encodings; peel extra waits onto same-engine NoOps.
        cnt = 0
        for f in nc.m.functions:
            for bb in f.blocks:
                new_l = []
                for inst in bb.instructions:
                    si = inst.sync_info
                    ws = list(si.on_wait) if si is not None else []
                    if len(ws) > 1:
                        for w in ws[:-1]:
                            nop = mybir.InstNoOp(
                                name=f"{inst.name}_wsplit{cnt}", engine=inst.engine,
                                bass_nofuse=True,
                                sync_info=mybir.SyncInfo(on_wait=[w], on_update=[]))
                            nc.register_instruction(nop, overwrite=True)
                            new_l.append(nop)
                            cnt += 1
                        inst.sync_info = mybir.SyncInfo(
                            on_wait=[ws[-1]], on_update=list(si.on_update))
                    new_l.append(inst)
                bb.instructions = new_l

    f32 = mybir.dt.float32
    f32r = mybir.dt.float32r
    AF = mybir.ActivationFunctionType
    OP = mybir.AluOpType

    nc = bass.Bass()
    qT_in = nc.dram_tensor("queryT", [DIM, S], f32r, kind="ExternalInput")
    kT_in = nc.dram_tensor("keyT", [DIM, S], f32r, kind="ExternalInput")
    vT_in = nc.dram_tensor("valueT", [DIM, S], f32r, kind="ExternalInput")
    wq_in = nc.dram_tensor("wqT", [DIM, 256], f32r, kind="ExternalInput")
    wk_in = nc.dram_tensor("wkT", [DIM, 128], f32r, kind="ExternalInput")
    wv_in = nc.dram_tensor("wvT", [DIM, 128], f32r, kind="ExternalInput")
    wo_in = nc.dram_tensor("woT", [256, DIM], f32r, kind="ExternalInput")
    bq_in = nc.dram_tensor("bq_col", [128, 2], f32, kind="ExternalInput")
    bk_in = nc.dram_tensor("bk_col", [128, 1], f32, kind="ExternalInput")
    bv_in = nc.dram_tensor("bv_col", [128, 1], f32, kind="ExternalInput")
    cos_in = nc.dram_tensor("cosT", [128, S], f32, kind="ExternalInput")
    sin_in = nc.dram_tensor("sinT", [128, S], f32, kind="ExternalInput")
    mask_in = nc.dram_tensor("masks", [128, 4 * 512], f32, kind="ExternalInput")
    out_dram = nc.dram_tensor("partial", [S, DIM], f32, kind="ExternalOutput")

    qT_r = qT_in.rearrange("(co ci) s -> ci co s", ci=128)
    kT_r = kT_in.rearrange("(co ci) s -> ci co s", ci=128)
    vT_r = vT_in.rearrange("(co ci) s -> ci co s", ci=128)

    with tile.TileContext(nc) as tc:
        with (
            tc.tile_pool(name="const", bufs=1) as cpool,
            tc.tile_pool(name="stream", bufs=3) as spool,
            tc.tile_pool(name="work", bufs=2) as wpool,
            tc.tile_pool(name="acts", bufs=1) as apool,
            tc.tile_pool(name="ps1", bufs=1, space="PSUM") as ps1,
            tc.tile_pool(name="ps2", bufs=2, space="PSUM") as ps2,
        ):
            # ---- constants / weights (loaded once)
            wq_sb = cpool.tile([128, 16, 256], f32r)
            nc.sync.dma_start(wq_sb[:], wq_in.rearrange("(co ci) d -> ci co d", ci=128))
            wk_sb = cpool.tile([128, 16, 128], f32r)
            nc.sync.dma_start(wk_sb[:], wk_in.rearrange("(co ci) d -> ci co d", ci=128))
            wv_sb = cpool.tile([128, 16, 128], f32r)
            nc.sync.dma_start(wv_sb[:], wv_in.rearrange("(co ci) d -> ci co d", ci=128))
            wo_sb = cpool.tile([128, 2, DIM], f32r)
            nc.sync.dma_start(wo_sb[:], wo_in.rearrange("(h d) e -> d h e", d=128))
            bq_sb = cpool.tile([128, 2], f32)
            nc.sync.dma_start(bq_sb[:], bq_in[:])
            bk_sb = cpool.tile([128, 1], f32)
            nc.sync.dma_start(bk_sb[:], bk_in[:])
            bv_sb = cpool.tile([128, 1], f32)
            nc.sync.dma_start(bv_sb[:], bv_in[:])
            cos_sb = cpool.tile([128, S], f32)
            nc.sync.dma_start(cos_sb[:], cos_in[:])
            sin_sb = cpool.tile([128, S], f32)
            nc.sync.dma_start(sin_sb[:], sin_in[:])
            mask_sb = cpool.tile([128, 4, 512], f32)
            nc.sync.dma_start(mask_sb[:], mask_in.rearrange("p (r q) -> p r q", r=4))
            ones_f = cpool.tile([128, 128], f32)
            nc.vector.memset(ones_f[:], 1.0)
            ones_mat = cpool.tile([128, 128], f32r)
            nc.vector.tensor_copy(out=ones_mat[:], in_=ones_f[:])
            ident = cpool.tile([128, 128], f32)
            make_identity(nc, ident[:])

            # ---- persistent activations
            q_rot = [apool.tile([128, S], f32r, tag=f"qrot{h}", name=f"qrot{h}") for h in range(2)]
            k_rot = apool.tile([128, S], f32r, tag="krot")
            v_sb = apool.tile([128, S], f32r, tag="vsb")   # [k_local, (kt d)] v rows
            ctxT = [apool.tile([128, S], f32r, tag=f"ctx{h}", name=f"ctx{h}") for h in range(2)]

            def rope(dst, raw, sc):
                # dst = raw*cos + swap(raw)*sinMod, sinMod has the -1 on the
                # low half baked in host-side (rotate_half sign).
                ssl = slice(sc * 512, sc * 512 + 512)
                swp = wpool.tile([128, 512], f32, tag="ropeswp")
                nc.vector.tensor_copy(out=swp[0:64, :], in_=raw[64:128, :])
                nc.vector.tensor_copy(out=swp[64:128, :], in_=raw[0:64, :])
                tmp = wpool.tile([128, 512], f32, tag="ropetmp")
                nc.vector.tensor_tensor(tmp[:], swp[:], sin_sb[:, ssl], OP.mult)
                nc.vector.tensor_tensor(dst[:, ssl], raw[:], cos_sb[:, ssl], OP.mult)
                nc.vector.tensor_tensor(dst[:, ssl], dst[:, ssl], tmp[:], OP.add)

            # ---- projections: qT (2 heads), kT, vT — stream over (sc, cc)
            for sc in range(4):
                ssl = slice(sc * 512, sc * 512 + 512)
                pq0 = ps1.tile([128, 512], f32, tag="A")
                pq1 = ps1.tile([128, 512], f32, tag="B")
                pk = ps1.tile([128, 512], f32, tag="C")
                pv = ps1.tile([128, 512], f32, tag="D")
                for cc in range(16):
                    qt = spool.tile([128, 512], f32r, tag="qs")
                    nc.sync.dma_start(qt[:], qT_r[:, cc, ssl])
                    kt_ = spool.tile([128, 512], f32r, tag="ks")
                    nc.sync.dma_start(kt_[:], kT_r[:, cc, ssl])
                    vt = spool.tile([128, 512], f32r, tag="vs")
                    nc.sync.dma_start(vt[:], vT_r[:, cc, ssl])
                    st, sp = cc == 0, cc == 15
                    nc.tensor.matmul(pq0[:], wq_sb[:, cc, 0:128],
                                     qt[:], start=st, stop=sp)
                    nc.tensor.matmul(pq1[:], wq_sb[:, cc, 128:256],
                                     qt[:], start=st, stop=sp)
                    nc.tensor.matmul(pk[:], wk_sb[:, cc],
                                     kt_[:], start=st, stop=sp)
                    nc.tensor.matmul(pv[:], wv_sb[:, cc],
                                     vt[:], start=st, stop=sp)
                # bias + RoPE (q, k); bias + transpose (v)
                q0_raw = wpool.tile([128, 512], f32, tag="raw")
                nc.scalar.activation(q0_raw[:], pq0[:], AF.Identity, bias=bq_sb[:, 0:1])
                rope(q_rot[0], q0_raw, sc)
                q1_raw = wpool.tile([128, 512], f32, tag="raw")
                nc.scalar.activation(q1_raw[:], pq1[:], AF.Identity, bias=bq_sb[:, 1:2])
                rope(q_rot[1], q1_raw, sc)
                k_raw = wpool.tile([128, 512], f32, tag="raw")
                nc.scalar.activation(k_raw[:], pk[:], AF.Identity, bias=bk_sb[:])
                rope(k_rot, k_raw, sc)
                v_raw = wpool.tile([128, 512], f32, tag="raw")
                nc.scalar.activation(v_raw[:], pv[:], AF.Identity, bias=bv_sb[:])
                for j in range(4):
                    ptr = ps1.tile([128, 128], f32, tag="A")
                    nc.tensor.transpose(ptr[:], v_raw[:, j * 128:(j + 1) * 128], ident[:])
                    nc.vector.tensor_copy(
                        out=v_sb[:, (sc * 4 + j) * 128:(sc * 4 + j) * 128 + 128],
                        in_=ptr[:])

            # ---- attention per head, per 512-query chunk
            for h in range(2):
                for qc in range(4):
                    qsl = slice(qc * 512, qc * 512 + 512)
                    n_kt = 4 * (qc + 1)
                    attnT = apool.tile([128, 16, 512], f32r, tag="attnT")
                    for kt in range(n_kt):
                        pst = ps2.tile([128, 512], f32, tag="sT")
                        nc.tensor.matmul(
                            pst[:], k_rot[:, kt * 128:(kt + 1) * 128],
                            q_rot[h][:, qsl], start=True, stop=True)
                        r = kt - 4 * qc
                        if r >= 0:
                            nc.vector.tensor_tensor(pst[:], pst[:], mask_sb[:, r], OP.add)
                        nc.scalar.activation(attnT[:, kt], pst[:], AF.Exp, scale=float(SCALE))
                    psum = ps1.tile([128, 512], f32, tag="C")
                    pctx = ps1.tile([128, 512], f32, tag="B")
                    for kt in range(n_kt):
                        nc.tensor.matmul(psum[:], ones_mat[:],
                                         attnT[:, kt],
                                         start=kt == 0, stop=kt == n_kt - 1)
                        nc.tensor.matmul(pctx[:], v_sb[:, kt * 128:(kt + 1) * 128],
                                         attnT[:, kt],
                                         start=kt == 0, stop=kt == n_kt - 1)
                    bc_sb = wpool.tile([128, 512], f32, tag="bc")
                    nc.vector.reciprocal(out=bc_sb[:], in_=psum[:])
                    nc.vector.tensor_tensor(ctxT[h][:, qsl], pctx[:], bc_sb[:], OP.mult)

            # ---- out-proj partial: [S, DIM] = sum_h ctxT_h.T @ woT_h
            for st in range(16):
                stsl = slice(st * 128, st * 128 + 128)
                for ec in range(4):
                    esl = slice(ec * 512, ec * 512 + 512)
                    po = ps2.tile([128, 512], f32, tag="po")
                    nc.tensor.matmul(po[:], ctxT[0][:, stsl],
                                     wo_sb[:, 0, esl], start=True, stop=False)
                    nc.tensor.matmul(po[:], ctxT[1][:, stsl],
                                     wo_sb[:, 1, esl], start=False, stop=True)
                    ot = wpool.tile([128, 512], f32, tag="ot")
                    if ec % 2 == 0:
                        nc.vector.tensor_copy(out=ot[:], in_=po[:])
                    else:
                        nc.scalar.activation(ot[:], po[:], AF.Copy)
                    nc.sync.dma_start(out_dram[stsl, esl], ot[:])
    _split_multi_waits(nc)
    return nc


def kernel(query, key, value, Wq, bq, Wk, bk, Wv, bv, Wo, bo):
    from concourse.bass_utils import run_bass_kernel_spmd

    query = np.asarray(query, np.float32)
    key = np.asarray(key, np.float32)
    value = np.asarray(value, np.float32)
    B = query.shape[0]
    qT = np.ascontiguousarray(query.reshape(S, DIM).T)
    kT = np.ascontiguousarray(key.reshape(S, DIM).T)
    vT = np.ascontiguousarray(value.reshape(S, DIM).T)
    cosT, sinT = _rope_cos_sin_T()
    sinT = sinT.copy()
    sinT[0:64, :] *= -1.0  # rotate_half: low half gets -x2*sin
    sinT = np.ascontiguousarray(sinT)
    masks = _masks()

    if "nc" not in _F32R_CACHE:
        _F32R_CACHE["nc"] = _build_program()
    nc = _F32R_CACHE["nc"]

    in_maps = []
    for i in range(N_CORES):
        g = i // 2
        Wq_s = np.ascontiguousarray(np.asarray(Wq, np.float32)[256 * i:256 * (i + 1), :].T)
        Wk_s = np.ascontiguousarray(np.asarray(Wk, np.float32)[128 * g:128 * (g + 1), :].T)
        Wv_s = np.ascontiguousarray(np.asarray(Wv, np.float32)[128 * g:128 * (g + 1), :].T)
        Wo_s = np.ascontiguousarray(np.asarray(Wo, np.float32)[:, 256 * i:256 * (i + 1)].T)
        bq_c = np.ascontiguousarray(np.asarray(bq, np.float32)[256 * i:256 * (i + 1)].reshape(2, 128).T)
        bk_c = np.asarray(bk, np.float32)[128 * g:128 * (g + 1)].reshape(128, 1)
        bv_c = np.asarray(bv, np.float32)[128 * g:128 * (g + 1)].reshape(128, 1)
        in_maps.append({
            "queryT": qT, "keyT": kT, "valueT": vT,
            "wqT": Wq_s, "wkT": Wk_s, "wvT": Wv_s, "woT": Wo_s,
            "bq_col": bq_c, "bk_col": np.ascontiguousarray(bk_c),
            "bv_col": np.ascontiguousarray(bv_c),
            "cosT": cosT, "sinT": sinT, "masks": masks,
        })

    _F32R_CACHE["in_maps"] = in_maps
    globals()["_LAST_IN_MAPS"] = in_maps
    res = run_bass_kernel_spmd(nc, in_maps, list(range(N_CORES)))
    out = res.results[0]["partial"].astype(np.float32)
    for i in range(1, N_CORES):
        out = out + res.results[i]["partial"]
    out = out + np.asarray(bo, np.float32)[None, :]
    return out.reshape(B, S, DIM).astype(np.float32)



# revision 6
# speedup vs baseline: 1.6714x; 1.6714x over previous
"""GQA attention kernel for 8 Trainium2 NeuronCores.

Sharding: tensor-parallel over heads. Core i handles query heads (2i, 2i+1)
and KV head i//2. Out-proj is row-parallel: each core emits a partial
[S, DIM] output (bf16); the host sums the 8 partials and adds the bias.

v2 changes vs baseline:
- everything staged/computed in bf16 (fp32 PSUM accumulation) -> half DMA
- fused per-512-query-chunk pipeline: proj(sc) -> rope -> attn(sc) ->
  out-proj(sc), with chunk sc+1 inputs prefetched during attn/out-proj
- DMA split across both hwdge queues (SP + Activation engines)
- DRAM staging layouts are contiguous per partition (big DMA packets)
- softmax denominator reciprocal on [1,512] + PE broadcast (was 3.4us
  full-tile DVE reciprocal)
"""

import numpy as np
import ml_dtypes

DIM = 2048
Q_HEADS = 16
KV_HEADS = 4
HEAD_DIM = 128
S = 2048
MAX_LEN = 2048
ROPE_THETA = 10000.0
ROPE_FACTOR = 8.0
N_CORES = 8
SCALE = 1.0 / np.sqrt(HEAD_DIM)
NEG = -1.0e30
CS = 512          # query chunk size
NC_CH = S // CS   # 4 chunks

_F32R_CACHE = {}

BF = ml_dtypes.bfloat16


def _rope_cos_sin_T():
    d = HEAD_DIM
    seq_eff = max(S, MAX_LEN)
    base_adj = (ROPE_FACTOR * seq_eff / MAX_LEN - (ROPE_FACTOR - 1.0)) ** (d / (d - 2))
    adjusted_base = ROPE_THETA * base_adj
    inv_freq = 1.0 / adjusted_base ** (np.arange(0, d, 2, dtype=np.float32) / d)
    pos = np.arange(S, dtype=np.float32)
    freqs = pos[:, None] * inv_freq[None, :]
    emb = np.concatenate([freqs, freqs], axis=-1)  # [S, d]
    return (
        np.ascontiguousarray(np.cos(emb).T.astype(np.float32)),  # [d, S]
        np.ascontiguousarray(np.sin(emb).T.astype(np.float32)),
    )


def _masks():
    # additive masks for the 4 diagonal 128x512 blocks: block r covers keys
    # [128r, 128r+128) against queries [0, 512) within a 512-query chunk.
    k = np.arange(128)[:, None]
    q = np.arange(512)[None, :]
    m = np.zeros((128, 4, 512), np.float32)
    for r in range(4):
        m[:, r, :] = np.where(128 * r + k > q, NEG, 0.0).astype(np.float32)
    return np.ascontiguousarray(m.reshape(128, 4 * 512))


def _build_program():
    import concourse.bass as bass
    import concourse.tile as tile
    from concourse import mybir
    import bass_rust
    from concourse.vector_clock import ScopedClock
    from concourse.masks import make_identity

    # --- workaround: walrus CTRL instructions accept a single sync wait;
    # split the TileContext end-drain waits across one SP nop each.
    def _patched_drain_and_barrier(self, tick_clock, wait_clock):
        nop0 = self.nc.sync.nop(nofuse=True)
        wait_clock.add_sem_waits(nop0.ins, ScopedClock({None: tick_clock.global_clock}))
        si = nop0.ins.sync_info
        ws = list(si.on_wait) if si is not None else []
        if len(ws) > 1:
            nop0.ins.sync_info = bass_rust.SyncInfo(
                on_wait=ws[:1], on_update=list(si.on_update))
            for i in range(1, len(ws)):
                nop = self.nc.sync.nop(nofuse=True)
                nop.ins.sync_info = bass_rust.SyncInfo(on_wait=ws[i:i + 1], on_update=[])
        self.nc.sync.drain()
        self.nc.all_engine_barrier()
        popped = self.nc._tile_sem_poison_stack.pop()
        assert popped is self._sem_poison
        self.nc.clear_and_free_semaphores(list(self.sems.allocated().values()))
        self.nc.all_engine_barrier()

    tile.TileContext._drain_and_barrier = _patched_drain_and_barrier

    def _split_multi_waits(nc):
        # this walrus build accepts a single sync-wait slot on several
        # instruction encodings; peel extra waits onto same-engine NoOps.
        cnt = 0
        for f in nc.m.functions:
            for bb in f.blocks:
                new_l = []
                for inst in bb.instructions:
                    si = inst.sync_info
                    ws = list(si.on_wait) if si is not None else []
                    if len(ws) > 1:
                        for w in ws[:-1]:
                            nop = mybir.InstNoOp(
                                name=f"{inst.name}_wsplit{cnt}", engine=inst.engine,
                                bass_nofuse=True,
                                sync_info=mybir.SyncInfo(on_wait=[w], on_update=[]))
                            nc.register_instruction(nop, overwrite=True)
                            new_l.append(nop)
                            cnt += 1
                        inst.sync_info = mybir.SyncInfo(
                            on_wait=[ws[-1]], on_update=list(si.on_update))
                    new_l.append(inst)
                bb.instructions = new_l

    f32 = mybir.dt.float32
    bf16 = mybir.dt.bfloat16
    AF = mybir.ActivationFunctionType
    OP = mybir.AluOpType

    nc = bass.Bass()
    # inputs staged [ci(128 part), sc(4), co(16), s'(512)] so every DMA is
    # contiguous per partition
    q_in = nc.dram_tensor("q_st", [128, NC_CH, 16, CS], bf16, kind="ExternalInput")
    k_in = nc.dram_tensor("k_st", [128, NC_CH, 16, CS], bf16, kind="ExternalInput")
    v_in = nc.dram_tensor("v_st", [128, NC_CH, 16, CS], bf16, kind="ExternalInput")
    wq_in = nc.dram_tensor("wq_st", [128, 16, 256], bf16, kind="ExternalInput")
    wk_in = nc.dram_tensor("wk_st", [128, 16, 128], bf16, kind="ExternalInput")
    wv_in = nc.dram_tensor("wv_st", [128, 16, 128], bf16, kind="ExternalInput")
    wo_in = nc.dram_tensor("wo_st", [128, 2, DIM], bf16, kind="ExternalInput")
    bq_in = nc.dram_tensor("bq_col", [128, 2], f32, kind="ExternalInput")
    bk_in = nc.dram_tensor("bk_col", [128, 1], f32, kind="ExternalInput")
    bv_in = nc.dram_tensor("bv_col", [128, 1], f32, kind="ExternalInput")
    cos_in = nc.dram_tensor("cosT", [128, S], bf16, kind="ExternalInput")
    sin_in = nc.dram_tensor("sinT", [128, S], bf16, kind="ExternalInput")
    mask_in = nc.dram_tensor("masks", [128, 4 * CS], f32, kind="ExternalInput")
    out_dram = nc.dram_tensor("partial", [S, DIM], bf16, kind="ExternalOutput")

    with tile.TileContext(nc) as tc:
        with (
            tc.tile_pool(name="const", bufs=1) as cpool,
            tc.tile_pool(name="slab", bufs=2) as spool,
            tc.tile_pool(name="attn", bufs=2) as atpool,
            tc.tile_pool(name="acts", bufs=1) as apool,
            tc.tile_pool(name="work", bufs=2) as wpool,
            tc.tile_pool(name="ot", bufs=2) as otpool,
            tc.tile_pool(name="psp", bufs=1, space="PSUM") as ps_proj,
            tc.tile_pool(name="pss", bufs=2, space="PSUM") as ps_attn,
            tc.tile_pool(name="psa", bufs=1, space="PSUM") as ps_acc,
        ):
            # ---- constants / weights. sync queue: wq, wk, wv (needed first)
            wq_sb = cpool.tile([128, 16, 256], bf16)
            nc.sync.dma_start(wq_sb[:], wq_in[:])
            wk_sb = cpool.tile([128, 16, 128], bf16)
            nc.sync.dma_start(wk_sb[:], wk_in[:])
            wv_sb = cpool.tile([128, 16, 128], bf16)
            nc.sync.dma_start(wv_sb[:], wv_in[:])
            # scalar queue: biases, trig (needed at first rope), mask, wo
            bq_sb = cpool.tile([128, 2], f32)
            nc.scalar.dma_start(bq_sb[:], bq_in[:])
            bk_sb = cpool.tile([128, 1], f32)
            nc.scalar.dma_start(bk_sb[:], bk_in[:])
            bv_sb = cpool.tile([128, 1], f32)
            nc.scalar.dma_start(bv_sb[:], bv_in[:])
            cos_sb = cpool.tile([128, S], bf16)
            nc.scalar.dma_start(cos_sb[:], cos_in[:])
            sin_sb = cpool.tile([128, S], bf16)
            nc.scalar.dma_start(sin_sb[:], sin_in[:])
            mask_sb = cpool.tile([128, 4, CS], f32)
            nc.scalar.dma_start(mask_sb[:], mask_in.rearrange("p (r q) -> p r q", r=4))
            wo_sb = cpool.tile([128, 2, DIM], bf16)
            nc.scalar.dma_start(wo_sb[:], wo_in[:])

            ones_f = cpool.tile([128, 128], f32)
            nc.vector.memset(ones_f[:], 1.0)
            ones_mat = cpool.tile([128, 128], bf16)
            nc.vector.tensor_copy(out=ones_mat[:], in_=ones_f[:])
            ones_row = cpool.tile([1, 128], bf16)
            nc.vector.memset(ones_row[:], 1.0)
            ident_f = cpool.tile([128, 128], f32)
            make_identity(nc, ident_f[:])
            ident = cpool.tile([128, 128], bf16)
            nc.vector.tensor_copy(out=ident[:], in_=ident_f[:])

            # ---- persistent activations (full sequence)
            k_rot = apool.tile([128, S], bf16, tag="krot")
            v_sb = apool.tile([128, S], bf16, tag="vsb")
            # chunk-local
            q_rot = [apool.tile([128, CS], bf16, tag=f"qrot{h}", name=f"qrot{h}")
                     for h in range(2)]
            ctxT = [apool.tile([128, CS], bf16, tag=f"ctx{h}", name=f"ctx{h}")
                    for h in range(2)]

            # input slab prefetch machinery: half-chunks of 8 contraction
            # blocks; q/k/v stacked. alternate issue engine per half.
            slab_q = {}

            def load_half(sc, hh):
                eng = nc.sync if (2 * sc + hh) % 2 == 0 else nc.scalar
                t = spool.tile([128, 3, 8, CS], bf16, tag="slab")
                eng.dma_start(t[:, 0], q_in[:, sc, 8 * hh:8 * hh + 8, :])
                eng.dma_start(t[:, 1], k_in[:, sc, 8 * hh:8 * hh + 8, :])
                eng.dma_start(t[:, 2], v_in[:, sc, 8 * hh:8 * hh + 8, :])
                slab_q[(sc, hh)] = t

            load_half(0, 0)
            load_half(0, 1)

            def rope(dst, raw, sc):
                # dst = raw*cos + swap(raw)*sinMod; sinMod has -1 baked into
                # the low half host-side (rotate_half sign).
                ssl = slice(sc * CS, sc * CS + CS)
                swp = wpool.tile([128, CS], bf16, tag="ropeswp")
                nc.vector.tensor_copy(out=swp[0:64, :], in_=raw[64:128, :])
                nc.vector.tensor_copy(out=swp[64:128, :], in_=raw[0:64, :])
                tmp = wpool.tile([128, CS], bf16, tag="ropetmp")
                nc.vector.tensor_tensor(tmp[:], swp[:], sin_sb[:, ssl], OP.mult)
                nc.vector.tensor_tensor(dst[:], raw[:], cos_sb[:, ssl], OP.mult)
                nc.vector.tensor_tensor(dst[:], dst[:], tmp[:], OP.add)

            for sc in range(NC_CH):
                ssl = slice(sc * CS, sc * CS + CS)
                # ---- projections for chunk sc
                pq0 = ps_proj.tile([128, CS], f32, tag="pq0")
                pq1 = ps_proj.tile([128, CS], f32, tag="pq1")
                pk = ps_proj.tile([128, CS], f32, tag="pk")
                pv = ps_proj.tile([128, CS], f32, tag="pv")
                for hh in range(2):
                    slab = slab_q.pop((sc, hh))
                    for c8 in range(8):
                        cc = 8 * hh + c8
                        st_, sp_ = cc == 0, cc == 15
                        nc.tensor.matmul(pq0[:], wq_sb[:, cc, 0:128],
                                         slab[:, 0, c8], start=st_, stop=sp_)
                        nc.tensor.matmul(pq1[:], wq_sb[:, cc, 128:256],
                                         slab[:, 0, c8], start=st_, stop=sp_)
                        nc.tensor.matmul(pk[:], wk_sb[:, cc],
                                         slab[:, 1, c8], start=st_, stop=sp_)
                        nc.tensor.matmul(pv[:], wv_sb[:, cc],
                                         slab[:, 2, c8], start=st_, stop=sp_)
                    if sc + 1 < NC_CH:
                        load_half(sc + 1, hh)

                # bias + RoPE (q0, q1, k); bias + transpose (v)
                q0_raw = wpool.tile([128, CS], bf16, tag="raw")
                nc.scalar.activation(q0_raw[:], pq0[:], AF.Identity, bias=bq_sb[:, 0:1])
                rope(q_rot[0], q0_raw, sc)
                q1_raw = wpool.tile([128, CS], bf16, tag="raw")
                nc.scalar.activation(q1_raw[:], pq1[:], AF.Identity, bias=bq_sb[:, 1:2])
                rope(q_rot[1], q1_raw, sc)
                k_raw = wpool.tile([128, CS], bf16, tag="raw")
                nc.scalar.activation(k_raw[:], pk[:], AF.Identity, bias=bk_sb[:])
                k_dst = wpool.tile([128, CS], bf16, tag="kdst")
                rope(k_dst, k_raw, sc)
                nc.vector.tensor_copy(out=k_rot[:, ssl], in_=k_dst[:])
                v_raw = wpool.tile([128, CS], bf16, tag="raw")
                nc.scalar.activation(v_raw[:], pv[:], AF.Identity, bias=bv_sb[:])
                for j, vtag in enumerate(("pq0", "pq1", "pk", "pv")):
                    ptr = ps_proj.tile([128, 128], bf16, tag=vtag)
                    nc.tensor.transpose(ptr[:], v_raw[:, j * 128:(j + 1) * 128], ident[:])
                    nc.vector.tensor_copy(
                        out=v_sb[:, (sc * 4 + j) * 128:(sc * 4 + j) * 128 + 128],
                        in_=ptr[:])

                # ---- attention for chunk sc (both heads)
                n_kt = 4 * (sc + 1)
                for h in range(2):
                    attnT = atpool.tile([128, 16, CS], bf16, tag="attnT")
                    for kt in range(n_kt):
                        pst = ps_attn.tile([128, CS], f32, tag="sT")
                        nc.tensor.matmul(
                            pst[:], k_rot[:, kt * 128:(kt + 1) * 128],
                            q_rot[h][:], start=True, stop=True)
                        r = kt - 4 * sc
                        if r >= 0:
                            nc.vector.tensor_tensor(pst[:], pst[:], mask_sb[:, r], OP.add)
                        nc.scalar.activation(attnT[:, kt], pst[:], AF.Exp,
                                             scale=float(SCALE))
                    den = ps_acc.tile([128, CS], f32, tag="den")
                    pctx = ps_acc.tile([128, CS], f32, tag="ctx")
                    for kt in range(n_kt):
                        nc.tensor.matmul(den[:], ones_mat[:], attnT[:, kt],
                                         start=kt == 0, stop=kt == n_kt - 1)
                    for kt in range(n_kt):
                        nc.tensor.matmul(pctx[:], v_sb[:, kt * 128:(kt + 1) * 128],
                                         attnT[:, kt],
                                         start=kt == 0, stop=kt == n_kt - 1)
                    # reciprocal on one partition, broadcast via 1-row matmul
                    bc1 = wpool.tile([1, CS], bf16, tag="bc1")
                    with nc.allow_low_precision(reason="1/den broadcast via bf16 matmul"):
                        nc.vector.reciprocal(out=bc1[:], in_=den[0:1, :])
                    bcb = ps_attn.tile([128, CS], f32, tag="sT")
                    nc.tensor.matmul(bcb[:], ones_row[:], bc1[:], start=True, stop=True)
                    bcb_sb = wpool.tile([128, CS], bf16, tag="bcb")
                    nc.scalar.activation(bcb_sb[:], bcb[:], AF.Copy)
                    nc.vector.tensor_tensor(ctxT[h][:], pctx[:], bcb_sb[:], OP.mult)

                # ---- out-proj partial rows for chunk sc
                for st in range(4):
                    rows = slice(sc * CS + st * 128, sc * CS + st * 128 + 128)
                    po = [ps_attn.tile([128, CS], f32, tag="sT", name="po0"),
                          ps_attn.tile([128, CS], f32, tag="sT", name="po1"),
                          ps_acc.tile([128, CS], f32, tag="den", name="po2"),
                          ps_acc.tile([128, CS], f32, tag="ctx", name="po3")]
                    for h in range(2):
                        for ec in range(4):
                            esl = slice(ec * CS, ec * CS + CS)
                            nc.tensor.matmul(po[ec][:],
                                             ctxT[h][:, st * 128:st * 128 + 128],
                                             wo_sb[:, h, esl],
                                             start=h == 0, stop=h == 1)
                    ot = otpool.tile([128, DIM], bf16, tag="ot")
                    for ec in range(4):
                        esl = slice(ec * CS, ec * CS + CS)
                        if ec % 2 == 0:
                            nc.vector.tensor_copy(out=ot[:, esl], in_=po[ec][:])
                        else:
                            nc.scalar.activation(ot[:, esl], po[ec][:], AF.Copy)
                    eng = nc.sync if st % 2 == 0 else nc.scalar
                    eng.dma_start(out_dram[rows, :], ot[:])

    _split_multi_waits(nc)
    return nc


def _stage_qkv(x):
    # [S, DIM] f32 -> [ci(128), sc(4), co(16), s'(512)] bf16 contiguous
    return np.ascontiguousarray(
        x.reshape(NC_CH, CS, 16, 128).transpose(3, 0, 2, 1).astype(BF))


def kernel(query, key, value, Wq, bq, Wk, bk, Wv, bv, Wo, bo):
    from concourse.bass_utils import run_bass_kernel_spmd

    query = np.asarray(query, np.float32)
    B = query.shape[0]
    q_st = _stage_qkv(query.reshape(S, DIM))
    k_st = _stage_qkv(np.asarray(key, np.float32).reshape(S, DIM))
    v_st = _stage_qkv(np.asarray(value, np.float32).reshape(S, DIM))
    cosT, sinT = _rope_cos_sin_T()
    sinT = sinT.copy()
    sinT[0:64, :] *= -1.0  # rotate_half: low half gets -x2*sin
    cosT = np.ascontiguousarray(cosT.astype(BF))
    sinT = np.ascontiguousarray(sinT.astype(BF))
    masks = _masks()

    if "nc" not in _F32R_CACHE:
        _F32R_CACHE["nc"] = _build_program()
    nc = _F32R_CACHE["nc"]

    Wq_f = np.asarray(Wq, np.float32)
    Wk_f = np.asarray(Wk, np.float32)
    Wv_f = np.asarray(Wv, np.float32)
    Wo_f = np.asarray(Wo, np.float32)
    bq_f = np.asarray(bq, np.float32)
    bk_f = np.asarray(bk, np.float32)
    bv_f = np.asarray(bv, np.float32)

    in_maps = []
    for i in range(N_CORES):
        g = i // 2
        wq_st = np.ascontiguousarray(
            Wq_f[256 * i:256 * (i + 1), :].T.reshape(16, 128, 256)
            .transpose(1, 0, 2).astype(BF))
        wk_st = np.ascontiguousarray(
            Wk_f[128 * g:128 * (g + 1), :].T.reshape(16, 128, 128)
            .transpose(1, 0, 2).astype(BF))
        wv_st = np.ascontiguousarray(
            Wv_f[128 * g:128 * (g + 1), :].T.reshape(16, 128, 128)
            .transpose(1, 0, 2).astype(BF))
        wo_st = np.ascontiguousarray(
            Wo_f[:, 256 * i:256 * (i + 1)].T.reshape(2, 128, DIM)
            .transpose(1, 0, 2).astype(BF))
        bq_c = np.ascontiguousarray(bq_f[256 * i:256 * (i + 1)].reshape(2, 128).T)
        bk_c = np.ascontiguousarray(bk_f[128 * g:128 * (g + 1)].reshape(128, 1))
        bv_c = np.ascontiguousarray(bv_f[128 * g:128 * (g + 1)].reshape(128, 1))
        in_maps.append({
            "q_st": q_st, "k_st": k_st, "v_st": v_st,
            "wq_st": wq_st, "wk_st": wk_st, "wv_st": wv_st, "wo_st": wo_st,
            "bq_col": bq_c, "bk_col": bk_c, "bv_col": bv_c,
            "cosT": cosT, "sinT": sinT, "masks": masks,
        })

    _F32R_CACHE["in_maps"] = in_maps
    globals()["_LAST_IN_MAPS"] = in_maps
    res = run_bass_kernel_spmd(nc, in_maps, list(range(N_CORES)))
    out = res.results[0]["partial"].astype(np.float32)
    for i in range(1, N_CORES):
        out = out + res.results[i]["partial"].astype(np.float32)
    out = out + np.asarray(bo, np.float32)[None, :]
    return out.reshape(B, S, DIM).astype(np.float32)


# revision 10
# speedup vs baseline: 2.2240x; 1.3306x over previous
"""GQA attention kernel for 8 Trainium2 NeuronCores.

Sharding: tensor-parallel over heads. Core i handles query heads (2i, 2i+1)
and KV head i//2. Out-proj is row-parallel: each core emits a partial
[S, DIM] output (bf16); the host sums the 8 partials and adds the bias.

v2 changes vs baseline:
- everything staged/computed in bf16 (fp32 PSUM accumulation) -> half DMA
- fused per-512-query-chunk pipeline: proj(sc) -> rope -> attn(sc) ->
  out-proj(sc), with chunk sc+1 inputs prefetched during attn/out-proj
- DMA split across both hwdge queues (SP + Activation engines)
- DRAM staging layouts are contiguous per partition (big DMA packets)
- softmax denominator reciprocal on [1,512] + PE broadcast (was 3.4us
  full-tile DVE reciprocal)
"""

import numpy as np
import ml_dtypes

DIM = 2048
Q_HEADS = 16
KV_HEADS = 4
HEAD_DIM = 128
S = 2048
MAX_LEN = 2048
ROPE_THETA = 10000.0
ROPE_FACTOR = 8.0
N_CORES = 8
SCALE = 1.0 / np.sqrt(HEAD_DIM)
NEG = -1.0e30
CS = 512          # query chunk size
NC_CH = S // CS   # 4 chunks

_F32R_CACHE = {}

BF = ml_dtypes.bfloat16


def _rope_cos_sin_T():
    d = HEAD_DIM
    seq_eff = max(S, MAX_LEN)
    base_adj = (ROPE_FACTOR * seq_eff / MAX_LEN - (ROPE_FACTOR - 1.0)) ** (d / (d - 2))
    adjusted_base = ROPE_THETA * base_adj
    inv_freq = 1.0 / adjusted_base ** (np.arange(0, d, 2, dtype=np.float32) / d)
    pos = np.arange(S, dtype=np.float32)
    freqs = pos[:, None] * inv_freq[None, :]
    emb = np.concatenate([freqs, freqs], axis=-1)  # [S, d]
    return (
        np.ascontiguousarray(np.cos(emb).T.astype(np.float32)),  # [d, S]
        np.ascontiguousarray(np.sin(emb).T.astype(np.float32)),
    )


def _masks():
    # additive masks for the 4 diagonal 128x512 blocks: block r covers keys
    # [128r, 128r+128) against queries [0, 512) within a 512-query chunk.
    k = np.arange(128)[:, None]
    q = np.arange(512)[None, :]
    m = np.zeros((128, 4, 512), np.float32)
    for r in range(4):
        m[:, r, :] = np.where(128 * r + k > q, NEG, 0.0).astype(np.float32)
    return np.ascontiguousarray(m.reshape(128, 4 * 512))


def _build_program():
    import concourse.bass as bass
    import concourse.tile as tile
    from concourse import mybir
    import bass_rust
    from concourse.vector_clock import ScopedClock
    from concourse.masks import make_identity

    # --- workaround: walrus CTRL instructions accept a single sync wait;
    # split the TileContext end-drain waits across one SP nop each.
    def _patched_drain_and_barrier(self, tick_clock, wait_clock):
        nop0 = self.nc.sync.nop(nofuse=True)
        wait_clock.add_sem_waits(nop0.ins, ScopedClock({None: tick_clock.global_clock}))
        si = nop0.ins.sync_info
        ws = list(si.on_wait) if si is not None else []
        if len(ws) > 1:
            nop0.ins.sync_info = bass_rust.SyncInfo(
                on_wait=ws[:1], on_update=list(si.on_update))
            for i in range(1, len(ws)):
                nop = self.nc.sync.nop(nofuse=True)
                nop.ins.sync_info = bass_rust.SyncInfo(on_wait=ws[i:i + 1], on_update=[])
        self.nc.sync.drain()
        self.nc.all_engine_barrier()
        popped = self.nc._tile_sem_poison_stack.pop()
        assert popped is self._sem_poison
        self.nc.clear_and_free_semaphores(list(self.sems.allocated().values()))
        self.nc.all_engine_barrier()

    tile.TileContext._drain_and_barrier = _patched_drain_and_barrier

    def _split_multi_waits(nc):
        # this walrus build accepts a single sync-wait slot on several
        # instruction encodings; peel extra waits onto same-engine NoOps.
        cnt = 0
        for f in nc.m.functions:
            for bb in f.blocks:
                new_l = []
                for inst in bb.instructions:
                    si = inst.sync_info
                    ws = list(si.on_wait) if si is not None else []
                    if len(ws) > 1:
                        for w in ws[:-1]:
                            nop = mybir.InstNoOp(
                                name=f"{inst.name}_wsplit{cnt}", engine=inst.engine,
                                bass_nofuse=True,
                                sync_info=mybir.SyncInfo(on_wait=[w], on_update=[]))
                            nc.register_instruction(nop, overwrite=True)
                            new_l.append(nop)
                            cnt += 1
                        inst.sync_info = mybir.SyncInfo(
                            on_wait=[ws[-1]], on_update=list(si.on_update))
                    new_l.append(inst)
                bb.instructions = new_l

    f32 = mybir.dt.float32
    bf16 = mybir.dt.bfloat16
    AF = mybir.ActivationFunctionType
    OP = mybir.AluOpType

    nc = bass.Bass()
    # inputs staged [ci(128 part), sc(4), co(16), s'(512)] so every DMA is
    # contiguous per partition
    q_in = nc.dram_tensor("q_st", [128, NC_CH, 16, CS], bf16, kind="ExternalInput")
    k_in = nc.dram_tensor("k_st", [128, NC_CH, 16, CS], bf16, kind="ExternalInput")
    v_in = nc.dram_tensor("v_st", [128, NC_CH, 16, CS], bf16, kind="ExternalInput")
    wq_in = nc.dram_tensor("wq_st", [128, 16, 256], bf16, kind="ExternalInput")
    wk_in = nc.dram_tensor("wk_st", [128, 16, 128], bf16, kind="ExternalInput")
    wv_in = nc.dram_tensor("wv_st", [128, 16, 128], bf16, kind="ExternalInput")
    wo_in = nc.dram_tensor("wo_st", [128, 2, DIM], bf16, kind="ExternalInput")
    bq_in = nc.dram_tensor("bq_col", [128, 2], f32, kind="ExternalInput")
    bk_in = nc.dram_tensor("bk_col", [128, 1], f32, kind="ExternalInput")
    bv_in = nc.dram_tensor("bv_col", [128, 1], f32, kind="ExternalInput")
    cos_in = nc.dram_tensor("cosT", [128, S], bf16, kind="ExternalInput")
    sin_in = nc.dram_tensor("sinT", [128, S], bf16, kind="ExternalInput")
    mask_in = nc.dram_tensor("masks", [128, 4 * CS], f32, kind="ExternalInput")
    out_dram = nc.dram_tensor("partial", [S, DIM], bf16, kind="ExternalOutput")

    with tile.TileContext(nc) as tc:
        with (
            tc.tile_pool(name="const", bufs=1) as cpool,
            tc.tile_pool(name="slab", bufs=2) as spool,
            tc.tile_pool(name="attn", bufs=2) as atpool,
            tc.tile_pool(name="acts", bufs=1) as apool,
            tc.tile_pool(name="work", bufs=2) as wpool,
            tc.tile_pool(name="ot", bufs=2) as otpool,
            tc.tile_pool(name="psp", bufs=1, space="PSUM") as ps_proj,
            tc.tile_pool(name="pss", bufs=2, space="PSUM") as ps_attn,
            tc.tile_pool(name="psa", bufs=1, space="PSUM") as ps_acc,
        ):
            # ---- constants / weights. sync queue: wq, wk, wv (needed first)
            wq_sb = cpool.tile([128, 16, 256], bf16)
            nc.sync.dma_start(wq_sb[:], wq_in[:])
            wk_sb = cpool.tile([128, 16, 128], bf16)
            nc.sync.dma_start(wk_sb[:], wk_in[:])
            wv_sb = cpool.tile([128, 16, 128], bf16)
            nc.sync.dma_start(wv_sb[:], wv_in[:])
            # scalar queue: biases, trig (needed at first rope), mask, wo
            bq_sb = cpool.tile([128, 2], f32)
            nc.scalar.dma_start(bq_sb[:], bq_in[:])
            bk_sb = cpool.tile([128, 1], f32)
            nc.scalar.dma_start(bk_sb[:], bk_in[:])
            bv_sb = cpool.tile([128, 1], f32)
            nc.scalar.dma_start(bv_sb[:], bv_in[:])

            ones_f = cpool.tile([128, 128], f32)
            nc.vector.memset(ones_f[:], 1.0)
            ones_mat = cpool.tile([128, 128], bf16)
            nc.vector.tensor_copy(out=ones_mat[:], in_=ones_f[:])
            ident_f = cpool.tile([128, 128], f32)
            make_identity(nc, ident_f[:])
            ident = cpool.tile([128, 128], bf16)
            nc.vector.tensor_copy(out=ident[:], in_=ident_f[:])

            # ---- persistent activations (full sequence)
            k_rot = apool.tile([128, S], bf16, tag="krot")
            v_sb = apool.tile([128, S], bf16, tag="vsb")
            # chunk-local
            q_rot = [apool.tile([128, CS], bf16, tag=f"qrot{h}", name=f"qrot{h}")
                     for h in range(2)]
            ctxT = [apool.tile([128, CS], bf16, tag=f"ctx{h}", name=f"ctx{h}")
                    for h in range(2)]

            # input slab prefetch machinery: half-chunks of 8 contraction
            # blocks; q/k/v stacked. alternate issue engine per half.
            slab_q = {}

            def load_half(sc, hh):
                eng = nc.sync if (2 * sc + hh) % 2 == 0 else nc.scalar
                t = spool.tile([128, 3, 8, CS], bf16, tag="slab")
                eng.dma_start(t[:, 0], q_in[:, sc, 8 * hh:8 * hh + 8, :])
                eng.dma_start(t[:, 1], k_in[:, sc, 8 * hh:8 * hh + 8, :])
                eng.dma_start(t[:, 2], v_in[:, sc, 8 * hh:8 * hh + 8, :])
                slab_q[(sc, hh)] = t

            load_half(0, 0)
            load_half(0, 1)

            # bulkier constants after the first input slabs
            cos_sb = cpool.tile([128, S], bf16)
            nc.scalar.dma_start(cos_sb[:], cos_in[:])
            sin_sb = cpool.tile([128, S], bf16)
            nc.scalar.dma_start(sin_sb[:], sin_in[:])
            mask_sb = cpool.tile([128, 4, CS], f32)
            nc.scalar.dma_start(mask_sb[:], mask_in.rearrange("p (r q) -> p r q", r=4))
            wo_sb = cpool.tile([128, 2, DIM], bf16)
            nc.scalar.dma_start(wo_sb[:], wo_in[:])

            def rope(dst, raw, sc):
                # dst = raw*cos + swap(raw)*sinMod; sinMod has -1 baked into
                # the low half host-side (rotate_half sign).
                ssl = slice(sc * CS, sc * CS + CS)
                swp = wpool.tile([128, CS], bf16, tag="ropeswp")
                nc.vector.tensor_copy(out=swp[0:64, :], in_=raw[64:128, :])
                nc.vector.tensor_copy(out=swp[64:128, :], in_=raw[0:64, :])
                tmp = wpool.tile([128, CS], bf16, tag="ropetmp")
                nc.vector.tensor_tensor(tmp[:], swp[:], sin_sb[:, ssl], OP.mult)
                nc.vector.tensor_tensor(dst[:], raw[:], cos_sb[:, ssl], OP.mult)
                nc.vector.tensor_tensor(dst[:], dst[:], tmp[:], OP.add)

            for sc in range(NC_CH):
                ssl = slice(sc * CS, sc * CS + CS)
                # ---- projections for chunk sc
                pq0 = ps_proj.tile([128, CS], f32, tag="pq0")
                pq1 = ps_proj.tile([128, CS], f32, tag="pq1")
                pk = ps_proj.tile([128, CS], f32, tag="pk")
                pv = ps_proj.tile([128, CS], f32, tag="pv")
                slabs = [slab_q.pop((sc, 0)), slab_q.pop((sc, 1))]
                # per-tensor loops: chunk compute can start as soon as the
                # q part of the slab lands (k/v parts still in flight)
                for hh in range(2):
                    for c8 in range(8):
                        cc = 8 * hh + c8
                        st_, sp_ = cc == 0, cc == 15
                        nc.tensor.matmul(pq0[:], wq_sb[:, cc, 0:128],
                                         slabs[hh][:, 0, c8], start=st_, stop=sp_)
                        nc.tensor.matmul(pq1[:], wq_sb[:, cc, 128:256],
                                         slabs[hh][:, 0, c8], start=st_, stop=sp_)
                for hh in range(2):
                    for c8 in range(8):
                        cc = 8 * hh + c8
                        nc.tensor.matmul(pk[:], wk_sb[:, cc],
                                         slabs[hh][:, 1, c8], start=cc == 0, stop=cc == 15)
                for hh in range(2):
                    for c8 in range(8):
                        cc = 8 * hh + c8
                        nc.tensor.matmul(pv[:], wv_sb[:, cc],
                                         slabs[hh][:, 2, c8], start=cc == 0, stop=cc == 15)
                if sc + 1 < NC_CH:
                    load_half(sc + 1, 0)
                    load_half(sc + 1, 1)

                # bias + RoPE (q0, q1, k); bias + transpose (v)
                q0_raw = wpool.tile([128, CS], bf16, tag="raw")
                nc.scalar.activation(q0_raw[:], pq0[:], AF.Identity, bias=bq_sb[:, 0:1])
                rope(q_rot[0], q0_raw, sc)
                q1_raw = wpool.tile([128, CS], bf16, tag="raw")
                nc.scalar.activation(q1_raw[:], pq1[:], AF.Identity, bias=bq_sb[:, 1:2])
                rope(q_rot[1], q1_raw, sc)
                k_raw = wpool.tile([128, CS], bf16, tag="raw")
                nc.scalar.activation(k_raw[:], pk[:], AF.Identity, bias=bk_sb[:])
                k_dst = wpool.tile([128, CS], bf16, tag="kdst")
                rope(k_dst, k_raw, sc)
                nc.vector.tensor_copy(out=k_rot[:, ssl], in_=k_dst[:])
                v_raw = wpool.tile([128, CS], bf16, tag="raw")
                nc.scalar.activation(v_raw[:], pv[:], AF.Identity, bias=bv_sb[:])
                for j, vtag in enumerate(("pq0", "pq1", "pk", "pv")):
                    ptr = ps_proj.tile([128, 128], bf16, tag=vtag)
                    nc.tensor.transpose(ptr[:], v_raw[:, j * 128:(j + 1) * 128], ident[:])
                    nc.vector.tensor_copy(
                        out=v_sb[:, (sc * 4 + j) * 128:(sc * 4 + j) * 128 + 128],
                        in_=ptr[:])

                # ---- attention for chunk sc (both heads)
                n_kt = 4 * (sc + 1)
                for h in range(2):
                    attnT = atpool.tile([128, 16, CS], bf16, tag="attnT")
                    for kt in range(n_kt):
                        pst = ps_attn.tile([128, CS], f32, tag="sT")
                        nc.tensor.matmul(
                            pst[:], k_rot[:, kt * 128:(kt + 1) * 128],
                            q_rot[h][:], start=True, stop=True)
                        r = kt - 4 * sc
                        if r >= 0:
                            nc.vector.tensor_tensor(pst[:], pst[:], mask_sb[:, r], OP.add)
                        nc.scalar.activation(attnT[:, kt], pst[:], AF.Exp,
                                             scale=float(SCALE))
                    den = ps_acc.tile([128, CS], f32, tag="den")
                    pctx = ps_acc.tile([128, CS], f32, tag="ctx")
                    for kt in range(n_kt):
                        nc.tensor.matmul(den[:], ones_mat[:], attnT[:, kt],
                                         start=kt == 0, stop=kt == n_kt - 1)
                    for kt in range(n_kt):
                        nc.tensor.matmul(pctx[:], v_sb[:, kt * 128:(kt + 1) * 128],
                                         attnT[:, kt],
                                         start=kt == 0, stop=kt == n_kt - 1)
                    # 1/den = exp(-ln(den)) on the Act engine: ln & exp share
                    # an activation table, so this is 2 cheap ops instead of
                    # the fixed-cost DVE reciprocal (3.3us)
                    den_ln = wpool.tile([128, CS], f32, tag="dln")
                    nc.scalar.activation(den_ln[:], den[:], AF.Ln)
                    den_inv = wpool.tile([128, CS], bf16, tag="dinv")
                    nc.scalar.activation(den_inv[:], den_ln[:], AF.Exp, scale=-1.0)
                    nc.vector.tensor_tensor(ctxT[h][:], pctx[:], den_inv[:], OP.mult)

                # ---- out-proj partial rows for chunk sc
                for st in range(4):
                    rows = slice(sc * CS + st * 128, sc * CS + st * 128 + 128)
                    po = [ps_attn.tile([128, CS], f32, tag="sT", name="po0"),
                          ps_attn.tile([128, CS], f32, tag="sT", name="po1"),
                          ps_acc.tile([128, CS], f32, tag="den", name="po2"),
                          ps_acc.tile([128, CS], f32, tag="ctx", name="po3")]
                    for h in range(2):
                        for ec in range(4):
                            esl = slice(ec * CS, ec * CS + CS)
                            nc.tensor.matmul(po[ec][:],
                                             ctxT[h][:, st * 128:st * 128 + 128],
                                             wo_sb[:, h, esl],
                                             start=h == 0, stop=h == 1)
                    ot = otpool.tile([128, DIM], bf16, tag="ot")
                    for ec in range(4):
                        esl = slice(ec * CS, ec * CS + CS)
                        if ec % 2 == 0:
                            nc.vector.tensor_copy(out=ot[:, esl], in_=po[ec][:])
                        else:
                            nc.scalar.activation(ot[:, esl], po[ec][:], AF.Copy)
                    eng = nc.sync if st % 2 == 0 else nc.scalar
                    eng.dma_start(out_dram[rows, :], ot[:])

    _split_multi_waits(nc)
    return nc


def _stage_qkv(x):
    # [S, DIM] f32 -> [ci(128), sc(4), co(16), s'(512)] bf16 contiguous
    return np.ascontiguousarray(
        x.reshape(NC_CH, CS, 16, 128).transpose(3, 0, 2, 1).astype(BF))


def kernel(query, key, value, Wq, bq, Wk, bk, Wv, bv, Wo, bo):
    from concourse.bass_utils import run_bass_kernel_spmd

    query = np.asarray(query, np.float32)
    B = query.shape[0]
    q_st = _stage_qkv(query.reshape(S, DIM))
    k_st = _stage_qkv(np.asarray(key, np.float32).reshape(S, DIM))
    v_st = _stage_qkv(np.asarray(value, np.float32).reshape(S, DIM))
    cosT, sinT = _rope_cos_sin_T()
    sinT = sinT.copy()
    sinT[0:64, :] *= -1.0  # rotate_half: low half gets -x2*sin
    cosT = np.ascontiguousarray(cosT.astype(BF))
    sinT = np.ascontiguousarray(sinT.astype(BF))
    masks = _masks()

    if "nc" not in _F32R_CACHE:
        _F32R_CACHE["nc"] = _build_program()
    nc = _F32R_CACHE["nc"]

    Wq_f = np.asarray(Wq, np.float32)
    Wk_f = np.asarray(Wk, np.float32)
    Wv_f = np.asarray(Wv, np.float32)
    Wo_f = np.asarray(Wo, np.float32)
    bq_f = np.asarray(bq, np.float32)
    bk_f = np.asarray(bk, np.float32)
    bv_f = np.asarray(bv, np.float32)

    in_maps = []
    for i in range(N_CORES):
        g = i // 2
        wq_st = np.ascontiguousarray(
            Wq_f[256 * i:256 * (i + 1), :].T.reshape(16, 128, 256)
            .transpose(1, 0, 2).astype(BF))
        wk_st = np.ascontiguousarray(
            Wk_f[128 * g:128 * (g + 1), :].T.reshape(16, 128, 128)
            .transpose(1, 0, 2).astype(BF))
        wv_st = np.ascontiguousarray(
            Wv_f[128 * g:128 * (g + 1), :].T.reshape(16, 128, 128)
            .transpose(1, 0, 2).astype(BF))
        wo_st = np.ascontiguousarray(
            Wo_f[:, 256 * i:256 * (i + 1)].T.reshape(2, 128, DIM)
            .transpose(1, 0, 2).astype(BF))
        bq_c = np.ascontiguousarray(bq_f[256 * i:256 * (i + 1)].reshape(2, 128).T)
        bk_c = np.ascontiguousarray(bk_f[128 * g:128 * (g + 1)].reshape(128, 1))
        bv_c = np.ascontiguousarray(bv_f[128 * g:128 * (g + 1)].reshape(128, 1))
        in_maps.append({
            "q_st": q_st, "k_st": k_st, "v_st": v_st,
            "wq_st": wq_st, "wk_st": wk_st, "wv_st": wv_st, "wo_st": wo_st,
            "bq_col": bq_c, "bk_col": bk_c, "bv_col": bv_c,
            "cosT": cosT, "sinT": sinT, "masks": masks,
        })

    _F32R_CACHE["in_maps"] = in_maps
    globals()["_LAST_IN_MAPS"] = in_maps
    res = run_bass_kernel_spmd(nc, in_maps, list(range(N_CORES)))
    out = res.results[0]["partial"].astype(np.float32)
    for i in range(1, N_CORES):
        out = out + res.results[i]["partial"].astype(np.float32)
    out = out + np.asarray(bo, np.float32)[None, :]
    return out.reshape(B, S, DIM).astype(np.float32)
